# revision 27
# baseline (speedup 1.0000x reference)
"""3-layer GCN (gather + segment-sum + LayerNorm) on 8 Trainium2 NeuronCores.

Strategy (destination-sharded, per the sharding hint):
  - Core r owns destination nodes [r*NLOC, (r+1)*NLOC).
  - Per layer: local GEMM h@W -> CHUNKED bf16 AllGather of post-GEMM
    features (CH window-aligned chunks of local rows; chunk c's collective
    overlaps chunk c-1's gathers) -> per-edge dma_gather of source rows
    (round-robin over 4 SWDGE queues for 4x DMA-engine parallelism) ->
    matmul-based segmented reduction (one-hot S matrices built on-chip,
    PSUM accumulation per 128-destination window) -> LayerNorm (+ReLU) ->
    PE transpose to h^T for the next layer's GEMM.
  - Edges are partitioned host-side by the source's local-row chunk
    (src mod NLOC); each chunk's table is core-major so indices fit int16.
  - The edge stream is padded so all 8 cores share one program structure
    (per-window tile counts = max over cores).
"""

import numpy as np
import ml_dtypes

import concourse.bass as bass
import concourse.tile as tile
from concourse import bacc, mybir
from concourse import bass_utils

dt = mybir.dt

D_IN, D_H, D_OUT = 128, 128, 64
EPS = 1e-5

LAST_RESULTS = None           # test.py introspection hook


class Cfg:
    def __init__(self, N, E, ncores=8, G=4, CH=2):
        assert N % ncores == 0
        self.N, self.E, self.NCORES, self.G = N, E, ncores, G
        self.NLOC = N // ncores
        self.WP = 128
        self.NW = (self.NLOC + self.WP - 1) // self.WP
        self.NG = (self.NW + G - 1) // G
        self.CH = CH
        # window-granular chunk boundaries in local rows; chunk c covers
        # windows [wlo_c, whi_c) -> local rows [wlo_c*WP, min(whi_c*WP, NLOC))
        base, rem = self.NW // CH, self.NW % CH
        counts = [base + 1] * rem + [base] * (CH - rem)
        self.chunk_wins = []
        self.chunk_rows = []          # (row_start, rows) per chunk
        w0 = 0
        for cnt in counts:
            w1 = w0 + cnt
            r0 = w0 * self.WP
            r1 = min(w1 * self.WP, self.NLOC)
            self.chunk_wins.append((w0, w1))
            self.chunk_rows.append((r0, r1 - r0))
            w0 = w1
        assert ncores * max(r for _, r in self.chunk_rows) <= 32767


REAL = Cfg(50000, 800000)


def host_prep(cfg, nodePointer, edgeList, edge_rows):
    """Common (cross-core max-padded) group-level tile structure plus the
    per-core gather-index / group-relative-slot streams.

    Layout: per group g (G dst-windows), per source chunk c (chunked
    AllGather table): all windows' edges concatenated (window-major,
    table-index-sorted within a window), padded to a multiple of 128 only
    at the chunk end.  Slots are group-relative (dst - group_base + pad
    sentinel 1000).  A tile may span windows; each window w gets matmuls
    over the union (across cores) of tiles its edges can live in.

    Edges are partitioned by the source node's LOCAL row chunk
    (src mod NLOC), so chunk c's gathers only need the AllGather of each
    core's local rows [row_start_c, row_start_c+rows_c) -- table c, laid
    out core-major: idx = (src // NLOC) * rows_c + (src mod NLOC) - r0_c.
    """
    NC, NLOC, NW, WP, NG, G, CH = (cfg.NCORES, cfg.NLOC, cfg.NW, cfg.WP,
                                   cfg.NG, cfg.G, cfg.CH)
    ptr = np.asarray(nodePointer, dtype=np.int64)
    src_all = np.asarray(edgeList, dtype=np.int64)
    dst_all = np.asarray(edge_rows, dtype=np.int64)
    row_starts = np.array([r0 for r0, _ in cfg.chunk_rows], dtype=np.int64)

    per_win = {}        # (r, w) -> [(idx_c, slot_c) for each chunk]
    for r in range(NC):
        for w in range(NW):
            d0 = r * NLOC + w * WP
            d1 = r * NLOC + min((w + 1) * WP, NLOC)
            e0, e1 = int(ptr[d0]), int(ptr[d1])
            src = src_all[e0:e1]
            gbase = r * NLOC + (w // G) * G * WP
            slot = dst_all[e0:e1] - gbase          # group-relative [0, G*128)
            lm = src % NLOC
            cid = np.searchsorted(row_starts, lm, side="right") - 1
            entry = []
            for c, (r0c, rowsc) in enumerate(cfg.chunk_rows):
                m = cid == c
                ix = (src[m] // NLOC) * rowsc + (lm[m] - r0c)
                o = np.argsort(ix, kind="stable")
                entry.append((ix[o], slot[m][o]))
            per_win[r, w] = entry

    groups = []
    t_cursor = 0
    for g in range(NG):
        wins = list(range(g * G, min((g + 1) * G, NW)))
        gm = {"g": g, "wins": wins, "chunks": []}
        for c in range(CH):
            # per-core cumulative edge offsets per window within this chunk
            cum = np.zeros((NC, len(wins) + 1), dtype=np.int64)
            for r in range(NC):
                for k, w in enumerate(wins):
                    cum[r, k + 1] = cum[r, k] + len(per_win[r, w][c][0])
            nt_all = int(np.ceil(cum[:, -1].max() / 128))
            ranges = []
            for k, w in enumerate(wins):
                t_first = int((cum[:, k] // 128).min())
                t_last = int(np.ceil(cum[:, k + 1] / 128).max())
                if t_last <= t_first:
                    t_first, t_last = 0, min(1, nt_all)
                ranges.append((t_first, t_last))
            gm["chunks"].append({"t0": t_cursor, "nt": nt_all, "cum": cum,
                                 "ranges": ranges})
            t_cursor += nt_all
        groups.append(gm)
    T_total = t_cursor

    idx_stream = np.zeros((NC, T_total * 128), dtype=np.int16)
    slot_stream = np.full((NC, T_total * 128), 1000, dtype=np.int16)
    for r in range(NC):
        for gm in groups:
            for c in range(CH):
                h = gm["chunks"][c]
                o = h["t0"] * 128
                for k, w in enumerate(gm["wins"]):
                    ix, sl = per_win[r, w][c]
                    p = o + int(h["cum"][r, k])
                    idx_stream[r, p:p + len(ix)] = ix.astype(np.int16)
                    slot_stream[r, p:p + len(sl)] = sl.astype(np.int16)

    idx_wrapped = np.empty((NC, 128, T_total * 8), dtype=np.int16)
    slots_T = np.empty((NC, 128, T_total), dtype=np.float16)
    for r in range(NC):
        a = idx_stream[r].reshape(T_total * 8, 16).T       # [16, T*8]
        idx_wrapped[r] = np.tile(a, (8, 1))
        slots_T[r] = slot_stream[r].reshape(T_total, 128).T.astype(
            np.float16)

    return groups, T_total, idx_wrapped, slots_T


def build_program(cfg, groups, T_total, shared_ag=True, maxt=6, gath_bufs=None,
                  repeat=1, fake_ag=False, one_core=False, ablate=(), swq=4,
                  lag=0, sgen_bufs=None):
    NC, NLOC, NW, N, CH = cfg.NCORES, cfg.NLOC, cfg.NW, cfg.N, cfg.CH
    build_program._gq = 0
    if gath_bufs is None:
        gath_bufs = min(16, max(2, 96 // maxt))
    if sgen_bufs is None:
        sgen_bufs = gath_bufs
    nc = bacc.Bacc("TRN2", target_bir_lowering=False, debug=False,
                   num_devices=1 if one_core else NC, num_swdge_queues=swq)
    f32, bf16, i16 = dt.float32, dt.bfloat16, dt.int16

    featT_d = nc.dram_tensor("featT", [128, NLOC], f32, kind="ExternalInput")
    w_d = [nc.dram_tensor("w0", [128, 128], f32, kind="ExternalInput"),
           nc.dram_tensor("w1", [128, 128], f32, kind="ExternalInput"),
           nc.dram_tensor("w2", [128, 64], f32, kind="ExternalInput")]
    idx_d = nc.dram_tensor("idxs", [128, T_total * 8], i16, kind="ExternalInput")
    slots_d = nc.dram_tensor("slots", [128, T_total], dt.float16, kind="ExternalInput")
    iota_d = nc.dram_tensor("iota", [128, 512], dt.float16, kind="ExternalInput")
    ident_d = nc.dram_tensor("ident", [128, 128], f32, kind="ExternalInput")
    hout_d = nc.dram_tensor("hout", [NLOC, 64], f32, kind="ExternalOutput")

    ldims = [D_H, D_H, D_OUT]          # true output dims: 128, 128, 64
    ag_in = []
    ag_full = []
    for l in range(3):
        ag_in.append(nc.dram_tensor(f"agin{l}", [NW * 128, 128], bf16))
        kw = {"addr_space": "Shared"} if shared_ag else {}
        ag_full.append(nc.dram_tensor(f"agfull{l}", [N, 128], bf16, **kw))

    with tile.TileContext(nc) as tc:
        with (
            tc.tile_pool(name="res", bufs=1) as res,
            tc.tile_pool(name="gath", bufs=gath_bufs) as gath,
            tc.tile_pool(name="sgen", bufs=sgen_bufs) as sgen,
            tc.tile_pool(name="lnp", bufs=2) as lnp,
            tc.tile_pool(name="ps", bufs=3, space="PSUM") as psp,
            tc.tile_pool(name="pst", bufs=2, space="PSUM") as pstp,
            tc.tile_pool(name="gps", bufs=2, space="PSUM") as gpsp,
        ):
            # ---- resident tiles ----
            hT = res.tile([128, NW * 128], f32, tag="hT")
            xw = res.tile([128, NW, 128], bf16, tag="xw")
            w_sb = [res.tile([128, 128], f32, tag="w0", name="w0sb"),
                    res.tile([128, 128], f32, tag="w1", name="w1sb"),
                    res.tile([128, 64], f32, tag="w2", name="w2sb")]
            iota_sb = res.tile([128, 512], dt.float16, tag="iota")
            ident_sb = res.tile([128, 128], f32, tag="ident")
            idx_sb = res.tile([128, T_total * 8], i16, tag="idx")
            slots_sb = res.tile([128, T_total], dt.float16, tag="slots")
            eps_sb = res.tile([128, 1], f32, tag="eps")
            nc.vector.memset(eps_sb[:], EPS)

            nc.sync.dma_start(hT[:, :NLOC], featT_d[:, :])
            for l in range(3):
                nc.sync.dma_start(w_sb[l][:], w_d[l][:, :])
            nc.sync.dma_start(iota_sb[:], iota_d[:, :])
            nc.sync.dma_start(ident_sb[:], ident_d[:, :])
            nc.sync.dma_start(idx_sb[:], idx_d[:, :])
            nc.sync.dma_start(slots_sb[:], slots_d[:, :])

            def emit_gemm_ag(l, ci):
                """GEMM h@W for chunk ci's windows, store to DRAM staging,
                launch that chunk's AllGather.  Emitted inside the PREVIOUS
                layer's group loop (right after the hT transposes it needs)
                so the collective hides under remaining gathers."""
                d = ldims[l]
                w0, w1 = cfg.chunk_wins[ci]
                r0, rows = cfg.chunk_rows[ci]
                if d < 128:
                    nc.vector.memset(xw[:, w0:w1, d:128], 0)
                if ci == CH - 1 and NLOC < NW * 128:
                    nc.vector.memset(xw[:, NW - 1, :d], 0)
                for c in range(w0, w1):
                    m = 128 if c < NW - 1 else NLOC - 128 * (NW - 1)
                    ps = gpsp.tile([128, d], f32, tag="gemm")
                    nc.tensor.matmul(ps[:m, :], hT[:, c * 128:c * 128 + m],
                                     w_sb[l][:], start=True, stop=True)
                    nc.vector.tensor_copy(xw[:m, c, :d], ps[:m, :])
                view = ag_in[l][w0 * 128:w1 * 128, :].rearrange(
                    "(c p) f -> p c f", p=128)
                nc.sync.dma_start(view, xw[:, w0:w1, :])
                if fake_ag:
                    # timing-sim stand-in: local copy, no collective
                    nc.sync.dma_start(
                        ag_full[l][NC * r0:NC * r0 + rows, :],
                        ag_in[l][r0:r0 + rows, :])
                else:
                    nc.gpsimd.collective_compute(
                        "AllGather", mybir.AluOpType.bypass,
                        replica_groups=[list(range(NC))],
                        ins=[ag_in[l][r0:r0 + rows, :].opt()],
                        outs=[ag_full[l][NC * r0:NC * (r0 + rows), :].opt()],
                    )

            seq = [x for _ in range(repeat) for x in range(3)]
            for k, l in enumerate(seq):
                d = ldims[l]          # true feature width (LN/output width)
                mdt = bf16
                if k == 0:
                    for ci in range(CH):
                        emit_gemm_ag(l, ci)
                nxt = seq[k + 1] if k + 1 < len(seq) else None
                trig = {}             # group index -> next-layer chunks
                if nxt is not None:
                    for ci in range(CH):
                        gi_last = (cfg.chunk_wins[ci][1] - 1) // cfg.G
                        trig.setdefault(gi_last, []).append(ci)

                io_sb = iota_sb
                sl_sb = slots_sb

                # ---- gather issue: chunk c trails chunk c-1 by LAG groups
                # so a chunk's first gather rarely stalls the in-order Pool
                # queue on its collective, while group-0's later-chunk
                # gathers still issue before the buffer rings wrap (deadlock
                # safety: LAG*(CH-1) * instrs_per_group_chunk < gath_bufs) ----
                tilemap = {}        # absolute tile index -> (gb, rel)
                sched = sorted((gi + ci * lag, ci, gi) for ci in range(CH)
                               for gi in range(len(groups)))
                for _, ci, gi in sched:
                    gm = groups[gi]
                    r0, rows = cfg.chunk_rows[ci]
                    src = ag_full[l][NC * r0:NC * (r0 + rows), :]
                    if True:
                        h = gm["chunks"][ci]
                        nt_all = h["nt"]
                        if nt_all == 0:
                            continue
                        t0_all = h["t0"]
                        for sub0 in range(0, nt_all, maxt):
                            nt = min(maxt, nt_all - sub0)
                            t0 = t0_all + sub0
                            gb = gath.tile([128, nt, 128], bf16,
                                           tag=f"g{ci}", name=f"gb_{ci}")
                            if "gather" in ablate:
                                nc.vector.memset(gb[:, 0, 0:1], 0)
                            else:
                                nc.gpsimd.dma_gather(
                                    gb[:], src,
                                    idx_sb[:, t0 * 8:(t0 + nt) * 8],
                                    num_idxs=nt * 128, num_idxs_reg=nt * 128,
                                    elem_size=128, single_packet=False,
                                    queue_num=build_program._gq % swq)
                                build_program._gq += 1
                            for t in range(nt):
                                tilemap[t0 + t] = (gb, t)

                # ---- message passing + LN per group ----
                for gi, gm in enumerate(groups):
                    gw = len(gm["wins"])
                    smap = {}       # (ci, wi) -> (S, t_first)
                    for ci in range(CH):
                        h = gm["chunks"][ci]
                        nt_all = h["nt"]
                        if nt_all == 0:
                            continue
                        t0_all = h["t0"]
                        for wi in range(gw):
                            tf, tl = h["ranges"][wi]
                            rl = tl - tf
                            if rl <= 0:
                                continue
                            S = sgen.tile([128, rl, 128], bf16,
                                          tag=f"s{ci}", name=f"S_{ci}")
                            if "sgen" in ablate:
                                nc.vector.memset(S[:, 0, 0:1], 0)
                            else:
                                nc.vector.tensor_tensor(
                                    S[:],
                                    iota_sb[:, wi * 128:(wi + 1) * 128]
                                    .unsqueeze(1)
                                    .broadcast_to([128, rl, 128]),
                                    slots_sb[:, t0_all + tf:t0_all + tl]
                                    .unsqueeze(2)
                                    .broadcast_to([128, rl, 128]),
                                    op=mybir.AluOpType.is_equal)
                            smap[ci, wi] = (S, t0_all + tf)

                    ps = psp.tile([128, gw, 128], f32, tag="ps")
                    for wi, w in enumerate(gm["wins"]):
                        mms = []
                        for ci in range(CH):
                            if (ci, wi) not in smap:
                                continue
                            S, st0 = smap[ci, wi]
                            h = gm["chunks"][ci]
                            tf, tl = h["ranges"][wi]
                            for t in range(h["t0"] + tf, h["t0"] + tl):
                                gb, rel = tilemap[t]
                                mms.append((S, t - st0, gb, rel))
                        if "mm" in ablate:
                            mms = mms[:1]
                        assert mms, f"window {w} has no matmuls"
                        for k, (S, si, gb, ti) in enumerate(mms):
                            # stream only the true feature width (64 for l=2)
                            nc.tensor.matmul(ps[:, wi, :d], S[:, si, :],
                                             gb[:, ti, :d], start=(k == 0),
                                             stop=(k == len(mms) - 1))

                    # ---- LayerNorm (+ReLU) ----
                    psd = ps[:, :, 0:d]
                    ssum = lnp.tile([128, gw], f32, tag="sum")
                    nc.vector.tensor_reduce(ssum[:], psd,
                                            axis=mybir.AxisListType.X,
                                            op=mybir.AluOpType.add)
                    negmu = lnp.tile([128, gw], f32, tag="negmu")
                    nc.vector.tensor_scalar_mul(negmu[:], ssum[:], -1.0 / d)
                    # vector-engine LN (avoids per-window scalar activations
                    # and their activation-table reloads)
                    xm = lnp.tile([128, gw, d], f32, tag="xm")
                    nc.vector.tensor_tensor(
                        xm[:], psd,
                        negmu[:].unsqueeze(2).broadcast_to([128, gw, d]),
                        op=mybir.AluOpType.add)
                    sq = lnp.tile([128, gw, d], f32, tag="sq")
                    nc.vector.tensor_tensor(sq[:], xm[:], xm[:],
                                            op=mybir.AluOpType.mult)
                    vsum = lnp.tile([128, gw], f32, tag="vsum")
                    nc.vector.tensor_reduce(vsum[:], sq[:],
                                            axis=mybir.AxisListType.X,
                                            op=mybir.AluOpType.add)
                    std = lnp.tile([128, gw], f32, tag="std")
                    nc.scalar.activation(std[:], vsum[:],
                                         mybir.ActivationFunctionType.Sqrt,
                                         bias=eps_sb[:], scale=1.0 / d)
                    rstd = lnp.tile([128, gw], f32, tag="rstd")
                    nc.vector.reciprocal(rstd[:], std[:])
                    y = lnp.tile([128, gw, d], f32, tag="y")
                    nc.vector.tensor_tensor(
                        y[:], xm[:],
                        rstd[:].unsqueeze(2).broadcast_to([128, gw, d]),
                        op=mybir.AluOpType.mult)

                    if l < 2:
                        for wi, w in enumerate(gm["wins"]):
                            pst = pstp.tile([128, 128], f32, tag="pst")
                            nc.tensor.transpose(pst[:], y[:, wi, :],
                                                ident_sb[:])
                            nc.vector.tensor_scalar_max(
                                hT[:, w * 128:(w + 1) * 128], pst[:], 0.0)
                    else:
                        for wi, w in enumerate(gm["wins"]):
                            rows = 128 if w < NW - 1 else NLOC - 128 * (NW - 1)
                            nc.sync.dma_start(
                                hout_d[w * 128:w * 128 + rows, :],
                                y[:rows, wi, :])

                    # pipelined next-layer GEMM + chunked AllGather
                    for ci in trig.get(gi, []):
                        emit_gemm_ag(nxt, ci)

    nc.compile()
    return nc


def make_in_maps(cfg, feat, W0, W1, W2, idx_wrapped, slots_T):
    NC, NLOC = cfg.NCORES, cfg.NLOC
    feat = np.asarray(feat, dtype=np.float32)
    iota_np = np.tile(np.arange(512, dtype=np.float32), (128, 1)).astype(
        np.float16)
    ident_np = np.eye(128, dtype=np.float32)
    in_maps = []
    for r in range(NC):
        in_maps.append({
            "featT": np.ascontiguousarray(feat[r * NLOC:(r + 1) * NLOC].T),
            "w0": np.asarray(W0, dtype=np.float32),
            "w1": np.asarray(W1, dtype=np.float32),
            "w2": np.asarray(W2, dtype=np.float32),
            "idxs": idx_wrapped[r],
            "slots": slots_T[r],
            "iota": iota_np,
            "ident": ident_np,
        })
    return in_maps


def kernel(feat, W0, W1, W2, nodePointer, edgeList, edge_rows):
    global LAST_RESULTS
    cfg = REAL
    groups, T_total, idx_wrapped, slots_T = host_prep(
        cfg, nodePointer, edgeList, edge_rows)
    nc = build_program(cfg, groups, T_total)
    in_maps = make_in_maps(cfg, feat, W0, W1, W2, idx_wrapped, slots_T)

    res = bass_utils.run_bass_kernel_spmd(
        nc, in_maps, core_ids=list(range(cfg.NCORES)))
    LAST_RESULTS = res
    out = np.concatenate([res.results[r]["hout"] for r in range(cfg.NCORES)],
                         axis=0)
    return out.astype(np.float32)

